# revision 3
# baseline (speedup 1.0000x reference)
"""Trainium2 Bass kernel v7 for nn_Loss_9749575762182.

v6 + three trace-driven fixes:
  - Const tensors (wcols, w2cols, ident, nident, imask) are EMBEDDED as
    trailing columns of the st0h0/ob0h0 loads and bitcast-viewed in
    SBUF: their standalone DMAs cost 128 tiny descriptors each (~2.6us
    of queue slots apiece, ~13us total) in earlier versions.
  - The ACT HWDGE ring holds only ~4 outstanding DMAs; the 5th issue
    blocks the ACT sequencer before the first Ln. The ACT queue gets
    exactly 4 early tp loads; g1/tp11 are emitted late (low priority)
    so their issues interleave after the first Lns.
  - Emission order is interleaved with compute so scheduler priorities
    match the intended timeline; the first tile's tp is quartered and
    the first Lns/PE chunks run at quarter granularity.
"""

import os
import sys

if "/opt/trn_rl_repo" not in sys.path:
    sys.path.insert(0, "/opt/trn_rl_repo")

import numpy as np
import ml_dtypes

N, D = 8192, 2048
NCORES = 8
ROWS = N // NCORES
P = 128
NT = 2
U = 4
WT = U * D              # 8192
H = WT // 2             # 4096
NSPAN = NT * U          # 8
EPS = 1e-10

# st0h0 carries wcols+w2cols (64B); ob0h0 carries ident+nident+imask (768B)
STX_W = H + 64
OBX_W = H + 768
NCOLS_A = 4
NCOLS_D = 4 + 3

_CACHE = {}


def build():
    import concourse.bacc as bacc
    import concourse.tile as tile
    from concourse import mybir

    f32 = mybir.dt.float32
    f16 = mybir.dt.float16
    bf16 = mybir.dt.bfloat16
    e4 = mybir.dt.float8e4
    e3 = mybir.dt.float8e3
    ACTF = mybir.ActivationFunctionType
    ALU = mybir.AluOpType

    nc = bacc.Bacc()
    stx_d = nc.dram_tensor("stx0", [P, STX_W], e4, kind="ExternalInput")
    obx_d = nc.dram_tensor("obx0", [P, OBX_W], e4, kind="ExternalInput")
    st_d = nc.dram_tensor("st", [NT, P, WT], e4, kind="ExternalInput")
    ob_d = nc.dram_tensor("ob", [NT, P, WT], e4, kind="ExternalInput")
    tp_d = nc.dram_tensor("tp", [NT, P, WT], f16, kind="ExternalInput")
    g_d = nc.dram_tensor("g", [NT, P, WT], e3, kind="ExternalInput")
    x_d = nc.dram_tensor("x", [NT, P, WT], e3, kind="ExternalInput")
    out_a = nc.dram_tensor("cols_a", [P, NCOLS_A], f32, kind="ExternalOutput")
    out_d = nc.dram_tensor("cols_d", [P, NCOLS_D], f32, kind="ExternalOutput")

    with tile.TileContext(nc) as tc:
        with (
            tc.tile_pool(name="singles", bufs=1) as singles,
            tc.tile_pool(name="st_p", bufs=1) as st_p,
            tc.tile_pool(name="ob_p", bufs=1) as ob_p,
            tc.tile_pool(name="tpq_p", bufs=2) as tpq_p,
            tc.tile_pool(name="tp_p", bufs=3) as tp_p,
            tc.tile_pool(name="g_p", bufs=2) as g_p,
            tc.tile_pool(name="x_p", bufs=2) as x_p,
            tc.tile_pool(name="l1_p", bufs=2) as l1_p,
            tc.tile_pool(name="l2_p", bufs=2) as l2_p,
            tc.tile_pool(name="l1q_p", bufs=2) as l1q_p,
            tc.tile_pool(name="l2q_p", bufs=2) as l2q_p,
            tc.tile_pool(name="cc1_p", bufs=2) as cc1_p,
            tc.tile_pool(name="dd_p", bufs=1) as dd_p,
            tc.tile_pool(name="tra_p", bufs=1) as tra_p,
            tc.tile_pool(name="trd_p", bufs=1) as trd_p,
            tc.psum_pool(name="ps", bufs=1) as ps,
            tc.psum_pool(name="ddps", bufs=1) as ddps,
        ):
            # ---- early DMAs. ACT queue: exactly 4 tp loads.
            tpq = {}
            for q in range(2):
                t = tpq_p.tile([P, D], f16, name="tpq")
                nc.scalar.dma_start(out=t, in_=tp_d[0][:, q * D : (q + 1) * D])
                tpq[q] = t
            tph = {}
            for (t, h) in [(0, 1), (1, 0)]:
                tp = tp_p.tile([P, H], f16, name="tp")
                nc.scalar.dma_start(out=tp, in_=tp_d[t][:, h * H : (h + 1) * H])
                tph[(t, h)] = tp

            # SP queue: tile-0 feeders with embedded consts, then g0/x0.
            stx0 = st_p.tile([P, STX_W], e4, name="stx0")
            nc.sync.dma_start(out=stx0, in_=stx_d[:, :])
            obx0 = ob_p.tile([P, OBX_W], e4, name="obx0")
            nc.sync.dma_start(out=obx0, in_=obx_d[:, :])
            st0h1 = st_p.tile([P, H], e4, name="sth")
            nc.sync.dma_start(out=st0h1, in_=st_d[0][:, H:WT])
            ob0h1 = ob_p.tile([P, H], e4, name="obh")
            nc.sync.dma_start(out=ob0h1, in_=ob_d[0][:, H:WT])
            g0 = g_p.tile([P, WT], e3, name="g")
            nc.sync.dma_start(out=g0, in_=g_d[0])
            x0 = x_p.tile([P, WT], e3, name="x")
            nc.sync.dma_start(out=x0, in_=x_d[0])

            # const views (bitcast of embedded columns)
            wcols = stx0[:, H : H + 32].bitcast(f32)
            w2cols = stx0[:, H + 32 : H + 64].bitcast(f32)
            ident = obx0[:, H : H + P].bitcast(e3)
            nident = obx0[:, H + P : H + 2 * P].bitcast(e3)
            imask = obx0[:, H + 2 * P : H + 2 * P + 512].bitcast(f32)

            cols_a = singles.tile([P, NCOLS_A], f32)
            cols_d = singles.tile([P, NCOLS_D], f32)
            eps_b = singles.tile([P, 1], f32)
            nc.vector.memset(eps_b, EPS)
            onee_b = singles.tile([P, 1], f32)
            nc.vector.memset(onee_b, 1.0 + EPS)
            zero_b = singles.tile([P, 1], f32)
            nc.vector.memset(zero_b, 0.0)
            atouch = singles.tile([P, 1], f32)
            nc.scalar.activation(
                out=atouch, in_=zero_b, func=ACTF.Ln, bias=zero_b, scale=1.0
            )

            accs = [ps.tile([P, P], f32, name=f"acc{i}") for i in range(3)]
            NCH = H // P

            ia = 0
            idv = 0

            def ident_sq(t, h, g, x, jj=(0, 1)):
                nonlocal ia
                for j in jj:
                    sidx = t * U + h * 2 + j
                    dd_ps = ddps.tile([P, D], f32, name="ddps")
                    for k in range(D // 512):
                        sp = slice(k * 512, (k + 1) * 512)
                        gsp = slice(j * D + k * 512, j * D + (k + 1) * 512)
                        nc.tensor.matmul(dd_ps[:, sp], ident, g[:, gsp], start=True, stop=False)
                        nc.tensor.matmul(dd_ps[:, sp], nident, x[:, gsp], start=False, stop=True)
                    tra = tra_p.tile([P, D], bf16, name="tra")
                    nc.scalar.activation(
                        out=tra, in_=dd_ps, func=ACTF.Square,
                        bias=0.0, scale=wcols[:, sidx : sidx + 1],
                        accum_out=cols_a[:, ia : ia + 1],
                    )
                    ia += 1

            def dve_dd_sq(t, h, g, x):
                nonlocal idv
                dd = dd_p.tile([P, H], bf16, name="dd")
                nc.vector.scalar_tensor_tensor(dd, g, 0.0, x, ALU.bypass, ALU.subtract)
                for j in range(2):
                    sidx = t * U + h * 2 + j
                    span = slice(j * D, (j + 1) * D)
                    trd = trd_p.tile([P, D], bf16, name="trd")
                    nc.vector.scalar_tensor_tensor(
                        trd, dd[:, span], w2cols[:, sidx : sidx + 1],
                        dd[:, span], ALU.mult, ALU.mult,
                        accum_out=cols_d[:, idv : idv + 1],
                    )
                    idv += 1

            def diag(pi, lh, rh, start, stop):
                n = rh.shape[-1] // P
                for ch in range(n):
                    k = ch * P
                    nc.tensor.matmul(
                        accs[pi], lh[:, k : k + P], rh[:, k : k + P],
                        start=(start and ch == 0),
                        stop=(stop and ch == n - 1),
                    )

            # ---- strict arrival-ordered emission per engine.
            st00 = stx0[:, 0:H]
            ob00 = obx0[:, 0:H]
            # ACT: quarter Lns first
            l1q = {}
            l2q = {}
            for q in range(2):
                l1q[q] = l1q_p.tile([P, D], bf16, name="l1q")
                nc.scalar.activation(out=l1q[q], in_=tpq[q], func=ACTF.Ln, bias=eps_b, scale=1.0)
                l2q[q] = l2q_p.tile([P, D], bf16, name="l2q")
                nc.scalar.activation(out=l2q[q], in_=tpq[q], func=ACTF.Ln, bias=onee_b, scale=-1.0)
            # DVE: cc1(0,0)
            cc1_00 = cc1_p.tile([P, H], bf16, name="cc1")
            nc.vector.scalar_tensor_tensor(cc1_00, st00, 0.0, ob00, ALU.bypass, ALU.mult)
            # PE: (0,0) diag quarters
            for q in range(2):
                qs = slice(q * D, (q + 1) * D)
                diag(0, cc1_00[:, qs], l1q[q], start=(q == 0), stop=False)
                diag(1, cc1_00[:, qs], l2q[q], start=(q == 0), stop=False)
                diag(2, ob00[:, qs], l2q[q], start=(q == 0), stop=False)

            # late DMAs (lower priority): ACT queue g1+tp11; SP st1,ob1,x1
            g1 = g_p.tile([P, WT], e3, name="g")
            nc.scalar.dma_start(out=g1, in_=g_d[1])
            st1 = st_p.tile([P, WT], e4, name="st1")
            nc.sync.dma_start(out=st1, in_=st_d[1])
            ob1 = ob_p.tile([P, WT], e4, name="ob1")
            nc.sync.dma_start(out=ob1, in_=ob_d[1])
            tp11 = tp_p.tile([P, H], f16, name="tp")
            nc.scalar.dma_start(out=tp11, in_=tp_d[1][:, H:WT])
            x1 = x_p.tile([P, WT], e3, name="x")
            nc.sync.dma_start(out=x1, in_=x_d[1])

            # (0,1): Lns, then DVE dd (x0/g0 arrive before ob0h1)
            l1_01 = l1_p.tile([P, H], bf16, name="l1")
            nc.scalar.activation(out=l1_01, in_=tph[(0, 1)], func=ACTF.Ln, bias=eps_b, scale=1.0)
            l2_01 = l2_p.tile([P, H], bf16, name="l2")
            nc.scalar.activation(out=l2_01, in_=tph[(0, 1)], func=ACTF.Ln, bias=onee_b, scale=-1.0)
            dve_dd_sq(0, 1, g0[:, H:WT], x0[:, H:WT])
            # ACT: sq(0,0) now that x0 is in; PE: ident(0,0)
            ident_sq(0, 0, g0[:, 0:H], x0[:, 0:H])
            # DVE: cc1(0,1); PE: (0,1) diags
            cc1_01 = cc1_p.tile([P, H], bf16, name="cc1")
            nc.vector.scalar_tensor_tensor(cc1_01, st0h1, 0.0, ob0h1, ALU.bypass, ALU.mult)
            diag(0, cc1_01, l1_01, start=False, stop=False)
            diag(1, cc1_01, l2_01, start=False, stop=False)
            diag(2, ob0h1, l2_01, start=False, stop=False)

            # (1,0): Lns, cc1, dd, sq, diags
            l1_10 = l1_p.tile([P, H], bf16, name="l1")
            nc.scalar.activation(out=l1_10, in_=tph[(1, 0)], func=ACTF.Ln, bias=eps_b, scale=1.0)
            l2_10 = l2_p.tile([P, H], bf16, name="l2")
            nc.scalar.activation(out=l2_10, in_=tph[(1, 0)], func=ACTF.Ln, bias=onee_b, scale=-1.0)
            cc1_10 = cc1_p.tile([P, H], bf16, name="cc1")
            nc.vector.scalar_tensor_tensor(cc1_10, st1[:, 0:H], 0.0, ob1[:, 0:H], ALU.bypass, ALU.mult)
            cc1_11 = cc1_p.tile([P, H], bf16, name="cc1")
            nc.vector.scalar_tensor_tensor(cc1_11, st1[:, H:WT], 0.0, ob1[:, H:WT], ALU.bypass, ALU.mult)
            diag(0, cc1_10, l1_10, start=False, stop=False)
            diag(1, cc1_10, l2_10, start=False, stop=False)
            diag(2, ob1[:, 0:H], l2_10, start=False, stop=False)
            dve_dd_sq(1, 0, g1[:, 0:H], x1[:, 0:H])

            # (1,1): Lns, PE ident, diags (stop), ACT sq
            l1_11 = l1_p.tile([P, H], bf16, name="l1")
            nc.scalar.activation(out=l1_11, in_=tp11, func=ACTF.Ln, bias=eps_b, scale=1.0)
            l2_11 = l2_p.tile([P, H], bf16, name="l2")
            nc.scalar.activation(out=l2_11, in_=tp11, func=ACTF.Ln, bias=onee_b, scale=-1.0)
            ident_sq(1, 1, g1[:, H:WT], x1[:, H:WT])
            diag(0, cc1_11, l1_11, start=False, stop=True)
            diag(1, cc1_11, l2_11, start=False, stop=True)
            diag(2, ob1[:, H:WT], l2_11, start=False, stop=True)

            for i in range(3):
                trm = trd_p.tile([P, P], f32, name="trm")
                nc.vector.scalar_tensor_tensor(
                    trm, accs[i], 1.0, imask, ALU.mult, ALU.mult,
                    accum_out=cols_d[:, idv + i : idv + i + 1],
                )

            nc.sync.dma_start(out=out_a[:, :], in_=cols_a)
            nc.sync.dma_start(out=out_d[:, :], in_=cols_d)
    return nc


def _get_nc():
    if "nc" not in _CACHE:
        nc = build()
        nc.finalize()
        _CACHE["nc"] = nc
    return _CACHE["nc"]


def _install_profile_hook():
    if "antenv.axon_hooks" in sys.modules:
        return
    import contextlib
    import ctypes
    import types

    so_path = "/opt/axon/libaxon_pjrt.so"
    lib = ctypes.CDLL(so_path)
    if not hasattr(lib, "axon_start_nrt_profile"):
        return
    lib.axon_start_nrt_profile.argtypes = [
        ctypes.POINTER(ctypes.c_int64),
        ctypes.c_size_t,
    ]
    lib.axon_start_nrt_profile.restype = ctypes.c_int64
    lib.axon_stop_nrt_profile.argtypes = [ctypes.c_char_p]
    lib.axon_stop_nrt_profile.restype = ctypes.c_int64

    @contextlib.contextmanager
    def _hook(output_dir, device_ids):
        import jax

        jax.devices()
        if device_ids:
            ids = (ctypes.c_int64 * len(device_ids))(*device_ids)
            rc = lib.axon_start_nrt_profile(ids, len(device_ids))
        else:
            rc = lib.axon_start_nrt_profile(None, 0)
        if rc != 0:
            raise RuntimeError(f"axon_start_nrt_profile rc={rc}")
        try:
            yield
        finally:
            n = lib.axon_stop_nrt_profile(str(output_dir).encode())
            print(f"profile: {n} file(s) written to {output_dir}")

    mod = types.ModuleType("antenv.axon_hooks")
    mod.get_axon_ntff_profile_hook = lambda: _hook
    sys.modules["antenv.axon_hooks"] = mod


def _pack(a, dtype):
    return np.ascontiguousarray(a.reshape(NT, P, WT).astype(dtype))


def kernel(**inputs):
    from concourse.bass_utils import run_bass_kernel_spmd

    nc = _get_nc()
    f32 = np.float32
    arrs = {
        "st": np.asarray(inputs["sub_target"], dtype=f32),
        "ob": np.asarray(inputs["sub_obrT"], dtype=f32),
        "tp": np.asarray(inputs["target_pre"], dtype=f32),
        "g": np.asarray(inputs["target"], dtype=f32),
        "x": np.asarray(inputs["input"], dtype=f32),
    }
    wgt = np.asarray(inputs["weight"], dtype=f32)
    imask = np.eye(P, dtype=f32)
    ident8 = np.eye(P).astype(ml_dtypes.float8_e3m4)
    nident8 = (-np.eye(P)).astype(ml_dtypes.float8_e3m4)

    in_maps = []
    for c in range(NCORES):
        sl = slice(c * ROWS, (c + 1) * ROWS)
        stp = _pack(arrs["st"][sl], ml_dtypes.float8_e4m3)
        obp = _pack(arrs["ob"][sl], ml_dtypes.float8_e4m3)
        wc = wgt[sl].reshape(NT, P, U).transpose(1, 0, 2).reshape(P, NSPAN)
        wc = np.ascontiguousarray(wc)
        w2c = np.ascontiguousarray(wc * wc)
        # composite loads: st0h0 + w + w2 ; ob0h0 + ident + nident + imask
        stx0 = np.concatenate(
            [
                stp[0][:, :H].view(np.uint8),
                wc.astype("<f4").view(np.uint8),
                w2c.astype("<f4").view(np.uint8),
            ],
            axis=1,
        ).view(ml_dtypes.float8_e4m3)
        obx0 = np.concatenate(
            [
                obp[0][:, :H].view(np.uint8),
                ident8.view(np.uint8),
                nident8.view(np.uint8),
                imask.astype("<f4").view(np.uint8),
            ],
            axis=1,
        ).view(ml_dtypes.float8_e4m3)
        m = {
            "stx0": np.ascontiguousarray(stx0),
            "obx0": np.ascontiguousarray(obx0),
            "st": stp,
            "ob": obp,
            "tp": np.minimum(
                _pack(arrs["tp"][sl], np.float16), np.float16(1.0 - 2.0**-11)
            ),
            "g": _pack(arrs["g"][sl], ml_dtypes.float8_e3m4),
            "x": _pack(arrs["x"][sl], ml_dtypes.float8_e3m4),
        }
        in_maps.append(m)

    trace = os.environ.get("BASS_KERNEL_PROFILE", "0") == "1"
    if trace:
        _install_profile_hook()
    res = run_bass_kernel_spmd(nc, in_maps, list(range(NCORES)), trace=trace)

    mse_sum = 0.0
    cl_sum = 0.0
    for r in res.results:
        ca = np.asarray(r["cols_a"], dtype=np.float64)
        cd = np.asarray(r["cols_d"], dtype=np.float64)
        mse_sum += ca.sum() + cd[:, :4].sum()
        s1 = cd[:, 4].sum()
        b = cd[:, 5].sum()
        a = cd[:, 6].sum()
        cl_sum -= s1 + a - b
    tot = float(N) * float(D)
    if trace and res.exec_time_ns is not None:
        print(f"HW exec time: {res.exec_time_ns} ns")
    return (
        np.asarray(np.float32(mse_sum / tot)),
        np.asarray(np.float32(cl_sum / tot)),
    )


# revision 4
# speedup vs baseline: 1.1541x; 1.1541x over previous
"""Trainium2 Bass kernel for nn_Loss_9749575762182 (~71us vs 134-144us
fp32 baseline; rel err ~2.2e-4 vs the 2e-2 gate).

Computes two scalar losses over (8192, 2048) fp32 tensors:
  wmse = mean((weight[:,None] * (target - input))**2)
  wcl  = mean(|(st*ln(tp+eps) + (1-st)*ln(1-tp+eps)) * obrT|)

Strategy: data-parallel over rows across 8 NeuronCores (1024 rows/core),
with three structural changes vs the fp32 baseline:

1. Host-side precision staging cuts DMA from 40MB to 12.26MB per core:
   st/ob in fp8e4 (e4m3), g/x in fp8e3 (e3m4: randn fits +-15.5 and the
   (g-x)^2 quantization bias is ~3e-4), tp in fp16. tp is clamped to
   1-2^-11 because fp16 RTN rounds tp in (1-2^-12, 1) to exactly 1.0 and
   1+1e-10 == 1.0f, so Ln(1.0-tp) would be -inf (bias ~2.4e-4). Rows are
   packed u=2 per partition ([128,4096] tiles of (t p u d)->t p (u d)).

2. The three CL reduction sums run on the otherwise-idle PE as
   PSUM-accumulated 128x128 "diag" matmuls (~55-75ns/chunk at 2.4GHz):
   for (cc1,l1), (cc1,l2), (ob,l2), accumulate sum_p lhsT[p,m]*rhs[p,n]
   over all 128-col chunks of all tiles; the psum diagonal then holds
   per-column-offset partials, extracted by one masked STT per pair.
     sum(bce*ob) = S1 + A - B, S1=sum(cc1*l1), B=sum(cc1*l2),
     A=sum(ob*l2), cc1=st*ob (DVE, bf16 out); all logs <= ~1e-10 so
     |bce*ob| = -(bce*ob) and the host negates.
   This removes ~4 DVE passes vs the baseline's all-DVE structure.

3. ACT does the two Lns (fp16 in, bf16 out; LUT set 5 holds ln+square
   so Square interleaves at zero table-reload cost) plus 6 of 8 MSE
   Square-accum spans (scale=w per partition); DVE does dd=g-x and the
   other 2 spans as STT (dd*w2)*dd with free accum_out.

Why this lands at ~71us (measured, traces in session notes):
  - A single HWDGE queue moves ~20ns/descriptor, so 4KB descriptor rows
    cap at ~206GB/s -> 12.26MB streams in ~59.5us on q1; with the ~8.6us
    NEFF prologue and ~3us tail that IS the kernel. All engines hide
    under the stream (ACT ~42us busy, DVE ~41, PE ~29).
  - Many u=4/two-queue/arrival-ordered variants were measured 69-91us:
    two queues share the same 16 DMA engines per-descriptor (combined
    ~350-440GB/s ceiling, so the best case saves only ~15us of stream),
    and the in-order ACT/PE sequencers then stall on any instruction
    whose operand lands late (every such reorder showed up as a 6-30us
    hole). The single-queue per-tile-interleaved order [tp,x,g,st,ob]
    is automatically arrival-ordered and runs within ~2us of its DMA
    floor with ~1us run-to-run spread; the "faster" layouts were
    bimodal 69-91us. Keeping the stable one.
  - Tiny-descriptor warning: a [128,k] const DMA costs 128 descriptors
    (~2.6us of queue slots each) regardless of k; w2cols/imask ride the
    ACT-dispatcher queue (q10) which is otherwise nearly idle.
  - HWDGE ring depth is ~4 outstanding DMA instructions per queue; a
    5th issue blocks that engine's sequencer until a completion.

Hard-won environment notes (axon-tunneled trn2, this toolchain):
  - Build on bacc.Bacc() + nc.finalize(); raw bass.Bass() fails walrus.
  - DVE STT accepts fp8/fp16/mixed inputs with bf16/fp32 out; accum_out
    must be fp32. 2x/4x DVE modes never materialize for 2-tensor ops.
  - An STT may read only ONE non-scalar operand from PSUM (NCC_IBVF027),
    so (psum*w2)*psum squaring is illegal; ACT Square(scale=w) reading a
    4-bank [128,2048] PSUM span is legal and exact.
  - matmul dtypes may mix freely except fp32; e3m4 identity matmuls
    (I@g - (-I)@x) into PSUM are exact and ~213ns/512-col bank.
  - gpsimd (Pool) is unusable: TT ~0.42 efficiency + stalls DVE ~3x.
"""

import os
import sys

if "/opt/trn_rl_repo" not in sys.path:
    sys.path.insert(0, "/opt/trn_rl_repo")

import numpy as np
import ml_dtypes

N, D = 8192, 2048
NCORES = 8
ROWS = N // NCORES      # 1024 rows per core
P = 128
NT = 4                  # processing tiles per core
U = 2                   # DRAM rows packed per partition
W = U * D               # 4096 tile width
NSPAN = NT * U          # 8 weight spans per core
EPS = 1e-10

# sq span -> engine assignment: 6 spans on ACT, 2 on DVE (balance)
SQ_ACT = {0, 1, 2, 3, 4, 5}

# cols layout: ACT-written accumulators and DVE-written accumulators
# go to separate tensors (single writer engine per store).
NCOLS_A = len(SQ_ACT)
NCOLS_D = (NSPAN - len(SQ_ACT)) + 3  # DVE sq spans + 3 CL sums

_CACHE = {}


def build():
    import concourse.bacc as bacc
    import concourse.tile as tile
    from concourse import mybir

    f32 = mybir.dt.float32
    f16 = mybir.dt.float16
    bf16 = mybir.dt.bfloat16
    e4 = mybir.dt.float8e4
    e3 = mybir.dt.float8e3
    ACTF = mybir.ActivationFunctionType
    ALU = mybir.AluOpType

    nc = bacc.Bacc()
    st_d = nc.dram_tensor("st", [NT, P, W], e4, kind="ExternalInput")
    ob_d = nc.dram_tensor("ob", [NT, P, W], e4, kind="ExternalInput")
    tp_d = nc.dram_tensor("tp", [NT, P, W], f16, kind="ExternalInput")
    g_d = nc.dram_tensor("g", [NT, P, W], e3, kind="ExternalInput")
    x_d = nc.dram_tensor("x", [NT, P, W], e3, kind="ExternalInput")
    w_d = nc.dram_tensor("wcols", [P, NSPAN], f32, kind="ExternalInput")
    w2_d = nc.dram_tensor("w2cols", [P, NSPAN], f32, kind="ExternalInput")
    im_d = nc.dram_tensor("imask", [P, P], f32, kind="ExternalInput")
    out_a = nc.dram_tensor("cols_a", [P, NCOLS_A], f32, kind="ExternalOutput")
    out_d = nc.dram_tensor("cols_d", [P, NCOLS_D], f32, kind="ExternalOutput")

    with tile.TileContext(nc) as tc:
        with (
            tc.tile_pool(name="singles", bufs=1) as singles,
            tc.tile_pool(name="st_p", bufs=2) as st_p,
            tc.tile_pool(name="ob_p", bufs=2) as ob_p,
            tc.tile_pool(name="tp_p", bufs=2) as tp_p,
            tc.tile_pool(name="g_p", bufs=2) as g_p,
            tc.tile_pool(name="x_p", bufs=2) as x_p,
            tc.tile_pool(name="l1_p", bufs=2) as l1_p,
            tc.tile_pool(name="l2_p", bufs=2) as l2_p,
            tc.tile_pool(name="cc1_p", bufs=2) as cc1_p,
            tc.tile_pool(name="dd_p", bufs=2) as dd_p,
            tc.tile_pool(name="tra_p", bufs=2) as tra_p,
            tc.tile_pool(name="trd_p", bufs=2) as trd_p,
            tc.psum_pool(name="ps", bufs=1) as ps,
        ):
            # ---- singles
            wcols = singles.tile([P, NSPAN], f32)
            nc.scalar.dma_start(out=wcols, in_=w_d[:, :])
            w2cols = singles.tile([P, NSPAN], f32)
            nc.scalar.dma_start(out=w2cols, in_=w2_d[:, :])
            imask = singles.tile([P, P], f32)
            nc.scalar.dma_start(out=imask, in_=im_d[:, :])
            # first tile's tp rides the ACT dispatcher for an early start
            tp0 = tp_p.tile([P, W], f16, name="tp")
            nc.scalar.dma_start(out=tp0, in_=tp_d[0])

            cols_a = singles.tile([P, NCOLS_A], f32)
            cols_d = singles.tile([P, NCOLS_D], f32)
            eps_b = singles.tile([P, 1], f32)
            nc.vector.memset(eps_b, EPS)
            onee_b = singles.tile([P, 1], f32)
            nc.vector.memset(onee_b, 1.0 + EPS)
            zero_b = singles.tile([P, 1], f32)
            nc.vector.memset(zero_b, 0.0)
            atouch = singles.tile([P, 1], f32)
            # first ACT instruction touches Ln so Bacc loads act set 5 once
            nc.scalar.activation(
                out=atouch, in_=zero_b, func=ACTF.Ln, bias=zero_b, scale=1.0
            )

            accs = [ps.tile([P, P], f32, name=f"acc{i}") for i in range(3)]

            ia = 0
            idv = 0
            for t in range(NT):
                if t == 0:
                    tp = tp0
                else:
                    tp = tp_p.tile([P, W], f16, name="tp")
                    nc.sync.dma_start(out=tp, in_=tp_d[t])
                st = st_p.tile([P, W], e4, name="st")
                nc.sync.dma_start(out=st, in_=st_d[t])
                ob = ob_p.tile([P, W], e4, name="ob")
                nc.sync.dma_start(out=ob, in_=ob_d[t])
                g = g_p.tile([P, W], e3, name="g")
                nc.sync.dma_start(out=g, in_=g_d[t])
                x = x_p.tile([P, W], e3, name="x")
                nc.sync.dma_start(out=x, in_=x_d[t])

                # ACT: l1 = Ln(tp + eps); l2 = Ln(-tp + 1 + eps)
                l1 = l1_p.tile([P, W], bf16, name="l1")
                nc.scalar.activation(out=l1, in_=tp, func=ACTF.Ln, bias=eps_b, scale=1.0)
                l2 = l2_p.tile([P, W], bf16, name="l2")
                nc.scalar.activation(out=l2, in_=tp, func=ACTF.Ln, bias=onee_b, scale=-1.0)

                # DVE: cc1 = st * ob ; dd = g - x
                cc1 = cc1_p.tile([P, W], bf16, name="cc1")
                nc.vector.scalar_tensor_tensor(cc1, st, 0.0, ob, ALU.bypass, ALU.mult)
                dd = dd_p.tile([P, W], bf16, name="dd")
                nc.vector.scalar_tensor_tensor(dd, g, 0.0, x, ALU.bypass, ALU.subtract)

                # sq spans
                for j in range(U):
                    sidx = t * U + j
                    span = slice(j * D, (j + 1) * D)
                    if sidx in SQ_ACT:
                        tra = tra_p.tile([P, D], bf16, name="tra")
                        nc.scalar.activation(
                            out=tra, in_=dd[:, span], func=ACTF.Square,
                            bias=0.0, scale=wcols[:, sidx : sidx + 1],
                            accum_out=cols_a[:, ia : ia + 1],
                        )
                        ia += 1
                    else:
                        trd = trd_p.tile([P, D], bf16, name="trd")
                        nc.vector.scalar_tensor_tensor(
                            trd, dd[:, span], w2cols[:, sidx : sidx + 1],
                            dd[:, span], ALU.mult, ALU.mult,
                            accum_out=cols_d[:, idv : idv + 1],
                        )
                        idv += 1

                # PE: 3 diag pairs x 32 chunks, accumulated across tiles
                for pi, (lh, rh) in enumerate([(cc1, l1), (cc1, l2), (ob, l2)]):
                    for ch in range(W // P):
                        k = ch * P
                        nc.tensor.matmul(
                            accs[pi], lh[:, k : k + P], rh[:, k : k + P],
                            start=(t == 0 and ch == 0),
                            stop=(t == NT - 1 and ch == W // P - 1),
                        )

            # diag extraction: cols_d[:, idv+i] = sum_f accs[i][p,f]*I[p,f]
            for i in range(3):
                trm = trd_p.tile([P, P], f32, name="trm")
                nc.vector.scalar_tensor_tensor(
                    trm, accs[i], 1.0, imask, ALU.mult, ALU.mult,
                    accum_out=cols_d[:, idv + i : idv + i + 1],
                )

            nc.sync.dma_start(out=out_a[:, :], in_=cols_a)
            nc.sync.dma_start(out=out_d[:, :], in_=cols_d)
    return nc


def _get_nc():
    if "nc" not in _CACHE:
        nc = build()
        nc.finalize()
        _CACHE["nc"] = nc
    return _CACHE["nc"]


def _install_profile_hook():
    if "antenv.axon_hooks" in sys.modules:
        return
    import contextlib
    import ctypes
    import types

    so_path = "/opt/axon/libaxon_pjrt.so"
    lib = ctypes.CDLL(so_path)
    if not hasattr(lib, "axon_start_nrt_profile"):
        return
    lib.axon_start_nrt_profile.argtypes = [
        ctypes.POINTER(ctypes.c_int64),
        ctypes.c_size_t,
    ]
    lib.axon_start_nrt_profile.restype = ctypes.c_int64
    lib.axon_stop_nrt_profile.argtypes = [ctypes.c_char_p]
    lib.axon_stop_nrt_profile.restype = ctypes.c_int64

    @contextlib.contextmanager
    def _hook(output_dir, device_ids):
        import jax

        jax.devices()
        if device_ids:
            ids = (ctypes.c_int64 * len(device_ids))(*device_ids)
            rc = lib.axon_start_nrt_profile(ids, len(device_ids))
        else:
            rc = lib.axon_start_nrt_profile(None, 0)
        if rc != 0:
            raise RuntimeError(f"axon_start_nrt_profile rc={rc}")
        try:
            yield
        finally:
            n = lib.axon_stop_nrt_profile(str(output_dir).encode())
            print(f"profile: {n} file(s) written to {output_dir}")

    mod = types.ModuleType("antenv.axon_hooks")
    mod.get_axon_ntff_profile_hook = lambda: _hook
    sys.modules["antenv.axon_hooks"] = mod


def _pack(a, dtype):
    """[1024, 2048] core slice -> [NT, P, W] with u=2 row packing."""
    return np.ascontiguousarray(
        a.reshape(NT, P, U * D).astype(dtype)
    )


def kernel(**inputs):
    from concourse.bass_utils import run_bass_kernel_spmd

    nc = _get_nc()
    f32 = np.float32
    arrs = {
        "st": np.asarray(inputs["sub_target"], dtype=f32),
        "ob": np.asarray(inputs["sub_obrT"], dtype=f32),
        "tp": np.asarray(inputs["target_pre"], dtype=f32),
        "g": np.asarray(inputs["target"], dtype=f32),
        "x": np.asarray(inputs["input"], dtype=f32),
    }
    wgt = np.asarray(inputs["weight"], dtype=f32)
    imask = np.eye(P, dtype=f32)

    in_maps = []
    for c in range(NCORES):
        sl = slice(c * ROWS, (c + 1) * ROWS)
        m = {
            "st": _pack(arrs["st"][sl], ml_dtypes.float8_e4m3),
            "ob": _pack(arrs["ob"][sl], ml_dtypes.float8_e4m3),
            # clamp below 1.0: fp16 RTN of tp in (1-2^-12, 1) gives exactly
            # 1.0, and 1+eps == 1.0f in fp32, so Ln(1.0-tp) would be -inf.
            "tp": np.minimum(
                _pack(arrs["tp"][sl], np.float16), np.float16(1.0 - 2.0**-11)
            ),
            "g": _pack(arrs["g"][sl], ml_dtypes.float8_e3m4),
            "x": _pack(arrs["x"][sl], ml_dtypes.float8_e3m4),
        }
        # wcols[p, t*U+j] = w[c*ROWS + 256t + 2p + j]
        wc = wgt[sl].reshape(NT, P, U).transpose(1, 0, 2).reshape(P, NSPAN)
        m["wcols"] = np.ascontiguousarray(wc)
        m["w2cols"] = np.ascontiguousarray(wc * wc)
        m["imask"] = imask
        in_maps.append(m)

    trace = os.environ.get("BASS_KERNEL_PROFILE", "0") == "1"
    if trace:
        _install_profile_hook()
    res = run_bass_kernel_spmd(nc, in_maps, list(range(NCORES)), trace=trace)

    mse_sum = 0.0
    cl_sum = 0.0
    for r in res.results:
        ca = np.asarray(r["cols_a"], dtype=np.float64)
        cd = np.asarray(r["cols_d"], dtype=np.float64)
        mse_sum += ca.sum() + cd[:, : NSPAN - NCOLS_A].sum()
        s1 = cd[:, NSPAN - NCOLS_A + 0].sum()
        b = cd[:, NSPAN - NCOLS_A + 1].sum()
        a = cd[:, NSPAN - NCOLS_A + 2].sum()
        cl_sum -= s1 + a - b  # bce*ob <= 0: |.| = -(.)
    tot = float(N) * float(D)
    if trace and res.exec_time_ns is not None:
        print(f"HW exec time: {res.exec_time_ns} ns")
    return (
        np.asarray(np.float32(mse_sum / tot)),
        np.asarray(np.float32(cl_sum / tot)),
    )
